# revision 4
# baseline (speedup 1.0000x reference)
"""Causal self-attention on 8 Trainium2 NeuronCores.

Problem: x[4,2048,1024] fp32, w_qkv[1024,3072], b_qkv[3072], w_out[1024,1024],
b_out[1024]; 16 heads, d_head 64; out = softmax_causal(QK^T/8) V @ w_out + b_out.

Sharding (hardcoded): core c handles batch b=c//2 and head-group g=c%2
(8 of 16 heads). Each core runs the full pipeline for its (batch, head-shard):
QKV projection, causal attention, and a partial output projection over its
512 head-channels. The host sums the two partial out-projections per batch
and adds b_out.

On-chip layout is "transposed": activations live as [channels, tokens] so
every matmul contracts over the partition dim. Scores are computed as
S^T[k,q] = K^T(stationary) @ Q^T per head with two heads packed into the
128-row PE array (row-tiled K=64 matmuls at partitions 0-63 / 64-127).
Softmax skips max-subtraction (scores are O(1) here), exp runs on the ACT
engine straight out of PSUM, causal masking is a bf16 0/1 multiply on the
four diagonal tiles, and the denominator comes free as a 65th ones-column of
V. Normalization: DVE reciprocal -> fp32r K=1 matmul broadcast -> DVE mul.

This container's walrus rejects >1 sync wait per instruction, so we
post-process the BIR JSON to hoist extra waits into standalone
EventSemaphore instructions (see _split_multi_waits_json).
"""

import json

import numpy as np
import ml_dtypes

import concourse.bass as bass
import concourse.mybir as mybir
from concourse.tile import TileContext
from concourse.bass_utils import run_bass_kernel_spmd

BF16 = ml_dtypes.bfloat16

# Set by test harnesses: trace=True captures NTFF profile; LAST_RESULTS holds
# the BassKernelResults of the most recent kernel() call.
TRACE = False
LAST_RESULTS = None

B, T, C = 4, 2048, 1024
H, DH = 16, 64
HL = 8  # heads per core
HDL = HL * DH  # 512 local head channels
QC = 512  # query-chunk width (PSUM bank limit for fp32 matmul out)
NQC = T // QC  # 4
NKT = T // 128  # 16 key tiles
N_CORES = 8

F32 = mybir.dt.float32
F32R = mybir.dt.float32r
BF = mybir.dt.bfloat16


def _split_multi_waits_json(raw: bytes) -> bytes:
    """Walrus here supports at most ONE sync wait per instruction. Hoist
    extras into standalone single-wait EventSemaphore instructions inserted
    immediately before, on the same engine (sequencers run in order, so
    waiting sequentially == waiting on all). Drains get ALL waits hoisted."""
    mod = json.loads(raw)
    ctr = 0
    for f in mod.get("functions", []):
        for blk in f.get("blocks", []):
            out = []
            changed = False
            for inst in blk.get("instructions", []):
                si = inst.get("sync_info")
                if si:
                    waits = si.get("on_wait") or []
                    keep = 0 if inst.get("opcode") == "Drain" else 1
                    if len(waits) > keep:
                        for w in waits[: len(waits) - keep]:
                            ctr += 1
                            out.append(
                                {
                                    "name": f"hoisted_wait_{ctr}",
                                    "engine": inst["engine"],
                                    "opcode": "EventSemaphore",
                                    "ins": [],
                                    "outs": [],
                                    "sync_info": {"on_wait": [w], "on_update": []},
                                }
                            )
                        si["on_wait"] = waits[len(waits) - keep :]
                        changed = True
                out.append(inst)
            if changed:
                blk["instructions"] = out
    return json.dumps(mod).encode()


def _build_nc() -> bass.Bass:
    nc = bass.Bass("TRN2", target_bir_lowering=False)

    xt_d = nc.dram_tensor("xt", [C, T], BF, kind="ExternalInput")
    wqk_d = nc.dram_tensor("wqk", [C, 1024], BF, kind="ExternalInput")
    bqk_d = nc.dram_tensor("bqk", [128, 8], F32, kind="ExternalInput")
    wv_d = nc.dram_tensor("wv", [C, HDL], BF, kind="ExternalInput")
    bv_d = nc.dram_tensor("bv", [1, HDL], BF, kind="ExternalInput")
    wout_d = nc.dram_tensor("wout", [HDL, C], BF, kind="ExternalInput")
    mask_d = nc.dram_tensor("mask", [4, 128, QC], BF, kind="ExternalInput")
    out_d = nc.dram_tensor("out_t", [C, T], F32, kind="ExternalOutput")

    exp_f = mybir.ActivationFunctionType.Exp

    with TileContext(nc) as tc:
        with (
            tc.tile_pool(name="consts", bufs=1) as consts,
            tc.tile_pool(name="ps_s", bufs=2, space="PSUM") as ps_s,
            tc.tile_pool(name="ps_y", bufs=2, space="PSUM") as ps_y,
            tc.tile_pool(name="ps_o", bufs=2, space="PSUM") as ps_o,
            tc.tile_pool(name="work", bufs=4) as work,
            tc.tile_pool(name="small", bufs=2) as small,
            tc.tile_pool(name="ostage", bufs=3) as ostage,
        ):
            xt_sb = [consts.tile([128, T], BF, name=f"xt_sb{i}") for i in range(8)]
            wqk_sb = [consts.tile([128, 1024], BF, name=f"wqk_sb{i}") for i in range(8)]
            wv_sb = [consts.tile([128, HDL], BF, name=f"wv_sb{i}") for i in range(8)]
            wout_sb = [consts.tile([128, C], BF, name=f"wout_sb{i}") for i in range(4)]
            bqk_sb = consts.tile([128, 8], F32, name="bqk_sb")
            bv_sb = consts.tile([1, HDL], BF, name="bv_sb")
            mask_sb = [consts.tile([128, QC], BF, name=f"mask_sb{r}") for r in range(4)]
            ones128 = consts.tile([1, 128], BF, name="ones128")
            ones64f = consts.tile([1, 64], F32, name="ones64f")
            ones64 = consts.tile([1, 64], F32R, name="ones64")
            qt_p = [consts.tile([128, T], BF, name=f"qt_pair{p}") for p in range(4)]
            kt_p = [consts.tile([128, T], BF, name=f"kt_pair{p}") for p in range(4)]
            vs = [consts.tile([128, HL, 65], BF, name=f"vs{t}") for t in range(NKT)]
            yt_p = [consts.tile([128, T], BF, name=f"yt_pair{p}") for p in range(4)]

            for i in range(8):
                nc.sync.dma_start(out=xt_sb[i], in_=xt_d[128 * i : 128 * (i + 1), :])
                nc.sync.dma_start(out=wqk_sb[i], in_=wqk_d[128 * i : 128 * (i + 1), :])
                nc.sync.dma_start(out=wv_sb[i], in_=wv_d[128 * i : 128 * (i + 1), :])
            for i in range(4):
                nc.sync.dma_start(out=wout_sb[i], in_=wout_d[128 * i : 128 * (i + 1), :])
                nc.sync.dma_start(out=mask_sb[i], in_=mask_d[i])
            nc.sync.dma_start(out=bqk_sb, in_=bqk_d[:, :])
            nc.sync.dma_start(out=bv_sb, in_=bv_d[:, :])
            nc.vector.memset(ones128, 1.0)
            nc.vector.memset(ones64f, 1.0)
            with nc.allow_low_precision(reason="exact 1.0 to f32r"):
                nc.vector.tensor_copy(out=ones64, in_=ones64f)
            for t in range(NKT):
                nc.vector.memset(vs[t][:, :, 64:65], 1.0)

            def qk_proj(mt):
                # mt 0-3: Q head-pairs, mt 4-7: K head-pairs
                dest = qt_p[mt] if mt < 4 else kt_p[mt - 4]
                for nch in range(NQC):
                    ps = ps_o.tile([128, QC], F32, tag="proj", name=f"psqk{mt}_{nch}")
                    for kt in range(8):
                        nc.tensor.matmul(
                            out=ps,
                            lhsT=wqk_sb[kt][:, 128 * mt : 128 * (mt + 1)],
                            rhs=xt_sb[kt][:, QC * nch : QC * (nch + 1)],
                            start=(kt == 0),
                            stop=(kt == 7),
                        )
                    nc.vector.tensor_scalar_add(
                        out=dest[:, QC * nch : QC * (nch + 1)],
                        in0=ps,
                        scalar1=bqk_sb[:, mt : mt + 1],
                    )

            def v_proj(tt):
                ps = ps_o.tile([128, HDL], F32, tag="proj", name=f"psv{tt}")
                for kt in range(8):
                    nc.tensor.matmul(
                        out=ps,
                        lhsT=xt_sb[kt][:, 128 * tt : 128 * (tt + 1)],
                        rhs=wv_sb[kt],
                        start=(kt == 0),
                        stop=False,
                    )
                nc.tensor.matmul(out=ps, lhsT=ones128, rhs=bv_sb, start=False, stop=True)
                nc.vector.tensor_copy(
                    out=vs[tt][:, :, 0:64],
                    in_=ps.rearrange("p (h d) -> p h d", h=HL),
                )

            def attention(qc, pair):
                n_kt = 4 * (qc + 1)  # causal: keys up to this q-chunk
                y_ps = [
                    ps_y.tile([65, QC], F32, tag="y", name=f"y{qc}_{pair}_{h}")
                    for h in (0, 1)
                ]
                for ktg in range(n_kt // 2):
                    kts = (2 * ktg, 2 * ktg + 1)
                    for half in (0, 1):
                        base = 64 * half
                        h = 2 * pair + half
                        s_ps = ps_s.tile(
                            [128, 2 * QC], F32, tag="s", name=f"s{qc}_{pair}_{ktg}_{half}"
                        )
                        for j, kt in enumerate(kts):
                            nc.tensor.matmul(
                                out=s_ps[:, QC * j : QC * (j + 1)],
                                lhsT=kt_p[pair][base : base + 64, 128 * kt : 128 * (kt + 1)],
                                rhs=qt_p[pair][base : base + 64, QC * qc : QC * (qc + 1)],
                                start=True,
                                stop=True,
                            )
                        ex = work.tile(
                            [128, 2 * QC], BF, tag="ex", name=f"ex{qc}_{pair}_{ktg}_{half}"
                        )
                        nc.scalar.activation(out=ex, in_=s_ps, func=exp_f, scale=0.125)
                        for j, kt in enumerate(kts):
                            r = kt - 4 * qc
                            if 0 <= r <= 3:
                                nc.vector.tensor_mul(
                                    ex[:, QC * j : QC * (j + 1)],
                                    ex[:, QC * j : QC * (j + 1)],
                                    mask_sb[r],
                                )
                        for j, kt in enumerate(kts):
                            nc.tensor.matmul(
                                out=y_ps[half],
                                lhsT=vs[kt][:, h, :],
                                rhs=ex[:, QC * j : QC * (j + 1)],
                                start=(kt == 0),
                                stop=(kt == n_kt - 1),
                            )
                for half in (0, 1):
                    base = 64 * half
                    r_sb = small.tile([1, QC], F32R, tag="r", name=f"r{qc}_{pair}_{half}")
                    with nc.allow_low_precision(reason="softmax denom recip"):
                        nc.vector.reciprocal(out=r_sb, in_=y_ps[half][64:65, :])
                    br = ps_s.tile([64, QC], F32, tag="s", name=f"br{qc}_{pair}_{half}")
                    nc.tensor.matmul(out=br, lhsT=ones64, rhs=r_sb, start=True, stop=True)
                    br_sb = work.tile(
                        [64, QC], F32, tag="brsb", bufs=2, name=f"brsb{qc}_{pair}_{half}"
                    )
                    nc.vector.tensor_copy(out=br_sb, in_=br)
                    nc.vector.tensor_mul(
                        out=yt_p[pair][base : base + 64, QC * qc : QC * (qc + 1)],
                        in0=y_ps[half][0:64, :],
                        in1=br_sb,
                    )

            def outproj(mt, nch):
                ps = ps_o.tile([128, QC], F32, tag="proj", name=f"pso{mt}_{nch}")
                for kt in range(4):
                    nc.tensor.matmul(
                        out=ps,
                        lhsT=wout_sb[kt][:, 128 * mt : 128 * (mt + 1)],
                        rhs=yt_p[kt][:, QC * nch : QC * (nch + 1)],
                        start=(kt == 0),
                        stop=(kt == 3),
                    )
                ob = ostage.tile([128, QC], F32, tag="ob", name=f"ob{mt}_{nch}")
                nc.vector.tensor_copy(out=ob, in_=ps)
                nc.sync.dma_start(
                    out=out_d[128 * mt : 128 * (mt + 1), QC * nch : QC * (nch + 1)],
                    in_=ob,
                )

            # Emission order interleaves PE-heavy projections with ACT-heavy
            # attention so the exp stream stays fed.
            qk_proj(0)
            qk_proj(4)
            for qc in range(NQC):
                for tt in range(4 * qc, 4 * qc + 4):
                    v_proj(tt)
                attention(qc, 0)
            for pair in (1, 2):
                qk_proj(pair)
                qk_proj(4 + pair)
                for qc in range(NQC):
                    attention(qc, pair)
            qk_proj(3)
            qk_proj(7)
            for qc in range(NQC):
                attention(qc, 3)
                for mt in range(8):
                    outproj(mt, qc)

    orig = nc.to_json_bytes
    nc.to_json_bytes = lambda: _split_multi_waits_json(orig())
    return nc


def _host_shards(x, w_qkv, b_qkv, w_out):
    """Per-core input dicts. Core c: batch c//2, head-group c%2."""
    mask = np.zeros((4, 128, QC), np.float32)
    kl = np.arange(128)[:, None]
    ql = np.arange(QC)[None, :]
    for r in range(4):
        mask[r] = (128 * r + kl) <= ql
    mask_h = np.ascontiguousarray(mask.astype(BF16))

    in_maps = []
    for c in range(N_CORES):
        b, g = divmod(c, 2)
        o = 512 * g
        w_q = w_qkv[:, o : o + 512]
        w_k = w_qkv[:, 1024 + o : 1024 + o + 512]
        w_v = w_qkv[:, 2048 + o : 2048 + o + 512]
        b_cat = np.concatenate([b_qkv[o : o + 512], b_qkv[1024 + o : 1024 + o + 512]])
        in_maps.append(
            {
                "xt": np.ascontiguousarray(x[b].T.astype(BF16)),
                "wqk": np.ascontiguousarray(
                    np.concatenate([w_q, w_k], axis=1).astype(BF16)
                ),
                "bqk": np.ascontiguousarray(
                    b_cat.reshape(8, 128).T.astype(np.float32)
                ),
                "wv": np.ascontiguousarray(w_v.astype(BF16)),
                "bv": np.ascontiguousarray(
                    b_qkv[2048 + o : 2048 + o + 512].reshape(1, 512).astype(BF16)
                ),
                "wout": np.ascontiguousarray(
                    w_out[512 * g : 512 * (g + 1), :].astype(BF16)
                ),
                "mask": mask_h,
            }
        )
    return in_maps


def kernel(x, w_qkv, b_qkv, w_out, b_out):
    global LAST_RESULTS
    x = np.asarray(x, dtype=np.float32)
    w_qkv = np.asarray(w_qkv, dtype=np.float32)
    b_qkv = np.asarray(b_qkv, dtype=np.float32)
    w_out = np.asarray(w_out, dtype=np.float32)
    b_out = np.asarray(b_out, dtype=np.float32)

    nc = _build_nc()
    in_maps = _host_shards(x, w_qkv, b_qkv, w_out)
    res = run_bass_kernel_spmd(
        nc, in_maps, core_ids=list(range(N_CORES)), trace=TRACE
    )
    LAST_RESULTS = res

    out = np.empty((B, T, C), np.float32)
    for b in range(B):
        p = res.results[2 * b]["out_t"] + res.results[2 * b + 1]["out_t"]
        out[b] = p.T + b_out[None, :]
    return out


# revision 15
# speedup vs baseline: 21.3447x; 21.3447x over previous
"""Causal self-attention on 8 Trainium2 NeuronCores.

Problem: x[4,2048,1024] fp32, w_qkv[1024,3072], b_qkv[3072], w_out[1024,1024],
b_out[1024]; 16 heads, d_head 64; out = softmax_causal(QK^T/8) V @ w_out + b_out.

Sharding (hardcoded): core c handles batch b=c//2 and head-group g=c%2
(8 of 16 heads). Each core runs the full pipeline for its (batch, head-shard):
QKV projection, causal attention, and a partial output projection over its
512 head-channels. The host sums the two partial out-projections per batch
and adds b_out.

On-chip layout is "transposed": activations live as [channels, tokens] so
every matmul contracts over the partition dim. Scores are computed as
S^T[k,q] = K^T(stationary) @ Q^T per head with two heads packed into the
128-row PE array (row-tiled K=64 matmuls at partitions 0-63 / 64-127).
Softmax skips max-subtraction (scores are O(1) here), exp runs on the ACT
engine straight out of PSUM, causal masking is a bf16 0/1 multiply on the
four diagonal tiles, and the denominator comes free as a 65th ones-column of
V. Normalization: DVE reciprocal -> fp32r K=1 matmul broadcast -> DVE mul.

This container's walrus rejects >1 sync wait per instruction, so we
post-process the BIR JSON to hoist extra waits into standalone
EventSemaphore instructions (see _split_multi_waits_json).
"""

import json

import numpy as np
import ml_dtypes

import concourse.bass as bass
import concourse.mybir as mybir
from concourse.tile import TileContext
from concourse.bass_utils import run_bass_kernel_spmd

BF16 = ml_dtypes.bfloat16

# Set by test harnesses: trace=True captures NTFF profile; LAST_RESULTS holds
# the BassKernelResults of the most recent kernel() call.
TRACE = False
LAST_RESULTS = None

# Benchmarking knob: emit the whole pipeline REPEAT times in one NEFF so
# T(2)-T(1) isolates one pipeline execution from dispatch overhead.
REPEAT = 1

B, T, C = 4, 2048, 1024
H, DH = 16, 64
HL = 8  # heads per core
HDL = HL * DH  # 512 local head channels
QC = 512  # query-chunk width (PSUM bank limit for fp32 matmul out)
NQC = T // QC  # 4
NKT = T // 128  # 16 key tiles
N_CORES = 8

F32 = mybir.dt.float32
F32R = mybir.dt.float32r
BF = mybir.dt.bfloat16


def _split_multi_waits_json(raw: bytes) -> bytes:
    """Walrus here supports at most ONE sync wait per instruction. Hoist
    extras into standalone single-wait EventSemaphore instructions inserted
    immediately before, on the same engine (sequencers run in order, so
    waiting sequentially == waiting on all). Drains get ALL waits hoisted."""
    mod = json.loads(raw)
    ctr = 0
    for f in mod.get("functions", []):
        for blk in f.get("blocks", []):
            out = []
            changed = False
            for inst in blk.get("instructions", []):
                si = inst.get("sync_info")
                if si:
                    waits = si.get("on_wait") or []
                    keep = 0 if inst.get("opcode") == "Drain" else 1
                    if len(waits) > keep:
                        for w in waits[: len(waits) - keep]:
                            ctr += 1
                            out.append(
                                {
                                    "name": f"hoisted_wait_{ctr}",
                                    "engine": inst["engine"],
                                    "opcode": "EventSemaphore",
                                    "ins": [],
                                    "outs": [],
                                    "sync_info": {"on_wait": [w], "on_update": []},
                                }
                            )
                        si["on_wait"] = waits[len(waits) - keep :]
                        changed = True
                out.append(inst)
            if changed:
                blk["instructions"] = out
    return json.dumps(mod).encode()


def _build_nc(with_v_bias: bool = True) -> bass.Bass:
    nc = bass.Bass("TRN2", target_bir_lowering=False)

    xt_d = nc.dram_tensor("xt", [C, T], BF, kind="ExternalInput")
    wqk_d = nc.dram_tensor("wqk", [C, 1024], BF, kind="ExternalInput")
    bqk_d = nc.dram_tensor("bqk", [128, 8], F32, kind="ExternalInput")
    wv_d = nc.dram_tensor("wv", [C, HDL], BF, kind="ExternalInput")
    bv_d = nc.dram_tensor("bv", [1, HDL], BF, kind="ExternalInput")
    wout_d = nc.dram_tensor("wout", [HDL, C], BF, kind="ExternalInput")
    mask_d = nc.dram_tensor("mask", [4, 128, QC], BF, kind="ExternalInput")
    out_d = nc.dram_tensor("out_t", [C, T], F32, kind="ExternalOutput")

    exp_f = mybir.ActivationFunctionType.Exp

    with TileContext(nc) as tc:
        with (
            tc.tile_pool(name="consts", bufs=1) as consts,
            tc.tile_pool(name="ps_s", bufs=2, space="PSUM") as ps_s,
            tc.tile_pool(name="ps_y", bufs=2, space="PSUM") as ps_y,
            tc.tile_pool(name="ps_o", bufs=2, space="PSUM") as ps_o,
            tc.tile_pool(name="work", bufs=4) as work,
            tc.tile_pool(name="small", bufs=2) as small,
            tc.tile_pool(name="ostage", bufs=3) as ostage,
        ):
            xt_sb = [consts.tile([128, T], BF, name=f"xt_sb{i}") for i in range(8)]
            wqk_sb = [consts.tile([128, 1024], BF, name=f"wqk_sb{i}") for i in range(8)]
            wv_sb = [consts.tile([128, HDL], BF, name=f"wv_sb{i}") for i in range(8)]
            wout_sb = [consts.tile([128, C], BF, name=f"wout_sb{i}") for i in range(4)]
            bqk_sb = consts.tile([128, 8], F32, name="bqk_sb")
            bv_sb = consts.tile([1, HDL], BF, name="bv_sb")
            mask_sb = [consts.tile([128, QC], BF, name=f"mask_sb{r}") for r in range(4)]
            ones128 = consts.tile([1, 128], BF, name="ones128")
            ones64f = consts.tile([1, 64], F32, name="ones64f")
            ones64 = consts.tile([1, 64], F32R, name="ones64")
            qt_p = [consts.tile([128, T], BF, name=f"qt_pair{p}") for p in range(4)]
            kt_p = [consts.tile([128, T], BF, name=f"kt_pair{p}") for p in range(4)]
            vs = [consts.tile([128, HL, 65], BF, name=f"vs{t}") for t in range(NKT)]
            yt_p = [consts.tile([128, T], BF, name=f"yt_pair{p}") for p in range(4)]

            # Load order: first chunk of every xt tile + all wqk so the first
            # projection chains start ASAP; bulk follows; wout (needed last)
            # goes last.
            for i in range(8):
                nc.sync.dma_start(out=wqk_sb[i], in_=wqk_d[128 * i : 128 * (i + 1), :])
                nc.sync.dma_start(
                    out=xt_sb[i][:, 0:QC], in_=xt_d[128 * i : 128 * (i + 1), 0:QC]
                )
            nc.sync.dma_start(out=bqk_sb, in_=bqk_d[:, :])
            for i in range(8):
                nc.sync.dma_start(out=wv_sb[i], in_=wv_d[128 * i : 128 * (i + 1), :])
            nc.sync.dma_start(out=bv_sb, in_=bv_d[:, :])
            for i in range(4):
                nc.sync.dma_start(out=mask_sb[i], in_=mask_d[i])
            for nch in range(1, 4):
                for i in range(8):
                    nc.sync.dma_start(
                        out=xt_sb[i][:, QC * nch : QC * (nch + 1)],
                        in_=xt_d[128 * i : 128 * (i + 1), QC * nch : QC * (nch + 1)],
                    )
            for i in range(4):
                nc.sync.dma_start(out=wout_sb[i], in_=wout_d[128 * i : 128 * (i + 1), :])
            nc.vector.memset(ones128, 1.0)
            nc.vector.memset(ones64f, 1.0)
            with nc.allow_low_precision(reason="exact 1.0 to f32r"):
                nc.vector.tensor_copy(out=ones64, in_=ones64f)
            for t in range(NKT):
                nc.vector.memset(vs[t][:, :, 64:65], 1.0)

            def qk_chunk(mt, nch):
                # mt 0-3: Q head-pairs, mt 4-7: K head-pairs
                dest = qt_p[mt] if mt < 4 else kt_p[mt - 4]
                ps = ps_o.tile([128, QC], F32, tag="proj", name=f"psqk{mt}_{nch}")
                for kt in range(8):
                    nc.tensor.matmul(
                        out=ps,
                        lhsT=wqk_sb[kt][:, 128 * mt : 128 * (mt + 1)],
                        rhs=xt_sb[kt][:, QC * nch : QC * (nch + 1)],
                        start=(kt == 0),
                        stop=(kt == 7),
                    )
                nc.vector.tensor_scalar_add(
                    out=dest[:, QC * nch : QC * (nch + 1)],
                    in0=ps,
                    scalar1=bqk_sb[:, mt : mt + 1],
                )

            def v_proj(tt):
                ps = ps_o.tile([128, HDL], F32, tag="proj", name=f"psv{tt}")
                for kt in range(8):
                    nc.tensor.matmul(
                        out=ps,
                        lhsT=xt_sb[kt][:, 128 * tt : 128 * (tt + 1)],
                        rhs=wv_sb[kt],
                        start=(kt == 0),
                        stop=(kt == 7 and not with_v_bias),
                    )
                if with_v_bias:
                    nc.tensor.matmul(
                        out=ps, lhsT=ones128, rhs=bv_sb, start=False, stop=True
                    )
                nc.vector.tensor_copy(
                    out=vs[tt][:, :, 0:64],
                    in_=ps.rearrange("p (h d) -> p h d", h=HL),
                )

            def attention(qc, pair):
                # Generator: yields after each k-tile group so filler PE work
                # can be woven between groups (keeps PE fed while ACT exps).
                n_kt = 4 * (qc + 1)  # causal: keys up to this q-chunk
                y_ps = [
                    ps_y.tile([65, QC], F32, tag="y", name=f"y{qc}_{pair}_{h}")
                    for h in (0, 1)
                ]
                for ktg in range(n_kt // 2):
                    kts = (2 * ktg, 2 * ktg + 1)
                    for half in (0, 1):
                        base = 64 * half
                        h = 2 * pair + half
                        s_ps = ps_s.tile(
                            [128, 2 * QC], F32, tag="s", name=f"s{qc}_{pair}_{ktg}_{half}"
                        )
                        for j, kt in enumerate(kts):
                            nc.tensor.matmul(
                                out=s_ps[:, QC * j : QC * (j + 1)],
                                lhsT=kt_p[pair][base : base + 64, 128 * kt : 128 * (kt + 1)],
                                rhs=qt_p[pair][base : base + 64, QC * qc : QC * (qc + 1)],
                                start=True,
                                stop=True,
                            )
                        ex = work.tile(
                            [128, 2 * QC],
                            BF,
                            tag="ex",
                            bufs=6,
                            name=f"ex{qc}_{pair}_{ktg}_{half}",
                        )
                        nc.scalar.activation(out=ex, in_=s_ps, func=exp_f, scale=0.125)
                        for j, kt in enumerate(kts):
                            r = kt - 4 * qc
                            if 0 <= r <= 3:
                                nc.vector.tensor_mul(
                                    ex[:, QC * j : QC * (j + 1)],
                                    ex[:, QC * j : QC * (j + 1)],
                                    mask_sb[r],
                                )
                        for j, kt in enumerate(kts):
                            nc.tensor.matmul(
                                out=y_ps[half],
                                lhsT=vs[kt][:, h, :],
                                rhs=ex[:, QC * j : QC * (j + 1)],
                                start=(kt == 0),
                                stop=(kt == n_kt - 1),
                            )
                    yield
                for half in (0, 1):
                    base = 64 * half
                    r_sb = small.tile([1, QC], F32R, tag="r", name=f"r{qc}_{pair}_{half}")
                    with nc.allow_low_precision(reason="softmax denom recip"):
                        nc.vector.reciprocal(out=r_sb, in_=y_ps[half][64:65, :])
                    br = ps_o.tile([64, QC], F32, tag="proj", name=f"br{qc}_{pair}_{half}")
                    nc.tensor.matmul(out=br, lhsT=ones64, rhs=r_sb, start=True, stop=True)
                    br_sb = work.tile(
                        [64, QC], F32, tag="brsb", bufs=2, name=f"brsb{qc}_{pair}_{half}"
                    )
                    nc.vector.tensor_copy(out=br_sb, in_=br)
                    nc.vector.tensor_mul(
                        out=yt_p[pair][base : base + 64, QC * qc : QC * (qc + 1)],
                        in0=y_ps[half][0:64, :],
                        in1=br_sb,
                    )

            def outproj(mt, nch):
                ps = ps_o.tile([128, QC], F32, tag="proj", name=f"pso{mt}_{nch}")
                for kt in range(4):
                    nc.tensor.matmul(
                        out=ps,
                        lhsT=wout_sb[kt][:, 128 * mt : 128 * (mt + 1)],
                        rhs=yt_p[kt][:, QC * nch : QC * (nch + 1)],
                        start=(kt == 0),
                        stop=(kt == 3),
                    )
                ob = ostage.tile([128, QC], F32, tag="ob", name=f"ob{mt}_{nch}")
                nc.vector.tensor_copy(out=ob, in_=ps)
                nc.sync.dma_start(
                    out=out_d[128 * mt : 128 * (mt + 1), QC * nch : QC * (nch + 1)],
                    in_=ob,
                )

            def weave(qc, pair, fillers):
                # Drive the attention generator, spreading filler emissions
                # evenly between its k-tile groups.
                g = attention(qc, pair)
                n = 2 * (qc + 1)
                m = len(fillers)
                done = 0
                for i in range(n):
                    next(g)
                    want = ((i + 1) * m) // n
                    while done < want:
                        fillers[done]()
                        done += 1
                for _ in g:  # tail (normalize) emission
                    pass
                while done < m:
                    fillers[done]()
                    done += 1

            def QK(mt, nch):
                return lambda: qk_chunk(mt, nch)

            def V(tt):
                return lambda: v_proj(tt)

            def OP(mt, nch):
                return lambda: outproj(mt, nch)

            # Filler schedule: each attention instance (pair, qc) carries the
            # PE-only work whose results are needed one-or-more instances
            # later, so PE never drains while ACT is the local bottleneck.
            fills = {
                (0, 0): [QK(0, 1), QK(4, 1), V(4), V(5), V(6), V(7)],
                (0, 1): [QK(0, 2), QK(4, 2), V(8), V(9), V(10), V(11)],
                (0, 2): [QK(0, 3), QK(4, 3), V(12), V(13), V(14), V(15)],
                (0, 3): [QK(1, 0), QK(5, 0), QK(1, 1), QK(5, 1)],
                (1, 0): [QK(1, 2), QK(5, 2)],
                (1, 1): [QK(1, 3), QK(5, 3)],
                (1, 2): [QK(2, 0), QK(6, 0)],
                (1, 3): [QK(2, 1), QK(6, 1), QK(2, 2), QK(6, 2)],
                (2, 0): [QK(2, 3), QK(6, 3)],
                (2, 1): [QK(3, 0), QK(7, 0)],
                (2, 2): [QK(3, 1), QK(7, 1)],
                (2, 3): [QK(3, 2), QK(7, 2), QK(3, 3), QK(7, 3)],
                (3, 0): [],
                (3, 1): [OP(mt, 0) for mt in range(8)],
                (3, 2): [OP(mt, 1) for mt in range(8)],
                (3, 3): [OP(mt, 2) for mt in range(8)],
            }

            for _rep in range(REPEAT):
                qk_chunk(0, 0)
                qk_chunk(4, 0)
                for tt in range(4):
                    v_proj(tt)
                for pair in range(4):
                    for qc in range(NQC):
                        weave(qc, pair, fills[(pair, qc)])
                for mt in range(8):
                    outproj(mt, 3)

    orig = nc.to_json_bytes
    nc.to_json_bytes = lambda: _split_multi_waits_json(orig())
    return nc


def _host_shards(x, w_qkv, b_qkv, w_out):
    """Per-core input dicts. Core c: batch c//2, head-group c%2."""
    mask = np.zeros((4, 128, QC), np.float32)
    kl = np.arange(128)[:, None]
    ql = np.arange(QC)[None, :]
    for r in range(4):
        mask[r] = (128 * r + kl) <= ql
    mask_h = np.ascontiguousarray(mask.astype(BF16))

    in_maps = []
    for c in range(N_CORES):
        b, g = divmod(c, 2)
        o = 512 * g
        w_q = w_qkv[:, o : o + 512]
        w_k = w_qkv[:, 1024 + o : 1024 + o + 512]
        w_v = w_qkv[:, 2048 + o : 2048 + o + 512]
        b_cat = np.concatenate([b_qkv[o : o + 512], b_qkv[1024 + o : 1024 + o + 512]])
        in_maps.append(
            {
                "xt": np.ascontiguousarray(x[b].T.astype(BF16)),
                "wqk": np.ascontiguousarray(
                    np.concatenate([w_q, w_k], axis=1).astype(BF16)
                ),
                "bqk": np.ascontiguousarray(
                    b_cat.reshape(8, 128).T.astype(np.float32)
                ),
                "wv": np.ascontiguousarray(w_v.astype(BF16)),
                "bv": np.ascontiguousarray(
                    b_qkv[2048 + o : 2048 + o + 512].reshape(1, 512).astype(BF16)
                ),
                "wout": np.ascontiguousarray(
                    w_out[512 * g : 512 * (g + 1), :].astype(BF16)
                ),
                "mask": mask_h,
            }
        )
    return in_maps


def kernel(x, w_qkv, b_qkv, w_out, b_out):
    global LAST_RESULTS
    x = np.asarray(x, dtype=np.float32)
    w_qkv = np.asarray(w_qkv, dtype=np.float32)
    b_qkv = np.asarray(b_qkv, dtype=np.float32)
    w_out = np.asarray(w_out, dtype=np.float32)
    b_out = np.asarray(b_out, dtype=np.float32)

    nc = _build_nc(with_v_bias=bool(np.any(b_qkv[2048:] != 0.0)))
    in_maps = _host_shards(x, w_qkv, b_qkv, w_out)
    res = run_bass_kernel_spmd(
        nc, in_maps, core_ids=list(range(N_CORES)), trace=TRACE
    )
    LAST_RESULTS = res

    out = np.empty((B, T, C), np.float32)
    for b in range(B):
        p = res.results[2 * b]["out_t"] + res.results[2 * b + 1]["out_t"]
        out[b] = p.T + b_out[None, :]
    return out
